# revision 18
# baseline (speedup 1.0000x reference)
"""Trainium2 Bass kernel for DLRA conv layer (3x3 low-rank conv + bias + relu).

Computes: relu(extract_patches_3x3(x) @ U @ W2 + bias) for the step-selected
factor set (W2 = S @ V folded on host for step 2). Sharded over H across 8
NeuronCores (28 rows each, 1-px halo resolved on host).

Device dataflow per core, per image:
  stage 1 (576->100): per quad (2 row-pairs A/B = 4 output rows, 896 px):
    - 3 full-array K=128 matmuls per row-pair contract shift pairs
      (di,0)+(di,1) via bufA = [x ; x<<1col] (2 HBM loads, no SBUF shift).
    - the 3 leftover dj=2 singles are K=64 and run as CONCURRENT row-tiled
      matmul pairs (tile_position (0,0)/(64,0)) -> 9 PE slots per 896 px.
    - ACT drains psum -> per-quad z1 tiles (fp16, [100 x 896]). ACT does
      NOTHING else in steady state: a busy ACT delays drains, drains gate
      stage-2, and stage-2 stalls are PE stalls (measured as the dominant
      mid-stream loss when ACT carried extra copies).
  stage 2 (100->256): one psum tile [128 x 1024] per (quad, filter-half),
    two N=448 matmuls at cols 0:448 / 512:960 (a matmul may not cross a
    psum bank). A ready-queue of drained quads is consumed down to a
    1-quad lag after each quad's stage-1.
  epilogue: bias-add reads PSUM directly on DVE (the only non-ACT engine
    with PSUM read access; gpsimd's vector ops are ~30x slower), then an
    in-place per-quad fp16 relu at DVE 4x mode. og strips store per fh in
    2 chunks (gpsimd queue for fh0, scalar queue for fh1). DVE runs ~95%
    busy; that is fine because the ps2 ring only needs an add every
    ~2.4us and DVE never falls a full tile behind.
  last image: per-quad stage-2 with immediate relu + store on rotating
    DMA queues; its fh1 tiles take the ACT fp32->fp16 copy + DVE 2x sbuf
    add path (ACT has slack once its drains are ending) so the post-
    matmul tail is one small tile deep (~4us instead of ~10us).

  Scheduling and DMA: each hardware DMA queue has ~2.6us bring-up and
  ~50-150GB/s effective early bandwidth, so the first ~16us are DMA-bound
  no matter what. x0's first 6 rows ride the sync queue alone, the rest
  splits sync (hi) / gpsimd (lo-shifted); ~26 dummy warm-up matmuls keep
  the PE HAM clock-gate counter alive until real work starts (the 2.4GHz
  clock needs ~3.4us of sustained PE activity, and any idle gap drops it
  to half speed for >=3.4us). w1 loads immediately; w2 + bias transfers
  are emitted behind quad-0's drain on the ACT stream so they cannot
  steal early HBM bandwidth from x0. The bias ships as fp8e4m3 (max abs
  quantization error ~8e-3 vs the 3.9e-2 tolerance) to halve its DMA
  cost; DVE upconverts it to fp16 chunk-by-chunk while nearly idle
  during image 0. Host transposes [fh,f,px] -> (H,W,256), casts fp32.
"""

import numpy as np
import ml_dtypes
from contextlib import ExitStack

import concourse.bacc as bacc
import concourse.tile as tile
import concourse.mybir as mybir
from concourse.bass_utils import run_bass_kernel_spmd

B, H, W, C = 8, 224, 224, 64
KH = KW = 3
RANK = 100
FILTERS = 256
IN_DIM = KH * KW * C  # 576

NCORES = 8
HS = H // NCORES          # 28 output rows per core
HSH = HS + 2              # input rows incl halo
WP = W + 2                # padded width
XL = HSH * WP             # flat image-strip length per channel (6780)
NPIX = HS * W             # 6272 pixels per image strip
RPP = 2 * W               # 448 px per row-pair (stage-1 matmul N)
QPX = 2 * RPP             # 896 px per quad
NQ = HS // 4              # 7 quads (2 row-pairs each) per image
MP = 128                  # padded stationary columns (rank 100 -> 128)
X0_ROWS = 10              # first-chunk padded rows of image 0
SCUT = 3 * QPX            # 2688: og store-chunk boundary

N_WARMUP_MM = 3
N_WARMDOWN_MM = 4

F32 = mybir.dt.float32
E4 = mybir.dt.float8e4
MM_DT = mybir.dt.float16
MM_NP = np.float16

_CACHE = {}


def _build_nc():
    nc = bacc.Bacc("TRN2", target_bir_lowering=False, debug=False,
                   num_devices=NCORES)
    xt = nc.dram_tensor("xt", [B, C, XL], MM_DT, kind="ExternalInput").ap()
    w1 = nc.dram_tensor("w1", [2 * C, 6 * MP], MM_DT,
                        kind="ExternalInput").ap()
    w2 = nc.dram_tensor("w2", [RANK, FILTERS], MM_DT,
                        kind="ExternalInput").ap()
    bias = nc.dram_tensor("bias", [MP, 2 * NPIX], E4,
                          kind="ExternalInput").ap()
    out = nc.dram_tensor("out", [B, 2, MP, NPIX], MM_DT,
                         kind="ExternalOutput").ap()
    fcopy = mybir.ActivationFunctionType.Copy
    add = mybir.AluOpType.add

    with tile.TileContext(nc) as tc, ExitStack() as ctx:
        const = ctx.enter_context(tc.tile_pool(name="const", bufs=1))
        xpool = ctx.enter_context(tc.tile_pool(name="xpool", bufs=2))
        z1pool = ctx.enter_context(tc.tile_pool(name="z1pool", bufs=4))
        ps1pool = ctx.enter_context(
            tc.tile_pool(name="ps1", bufs=2, space="PSUM"))
        ps2pool = ctx.enter_context(
            tc.tile_pool(name="ps2", bufs=2, space="PSUM"))
        ogpool = ctx.enter_context(tc.tile_pool(name="ogpool", bufs=4))
        t2pool = ctx.enter_context(tc.tile_pool(name="t2pool", bufs=2))

        w1_t = const.tile([2 * C, 6 * MP], MM_DT, name="w1_t")
        nc.scalar.dma_start(w1_t[:], w1[:])
        w2_t = const.tile([RANK, FILTERS], MM_DT, name="w2_t")
        bias8_t = const.tile([MP, 2 * NPIX], E4, name="bias8_t")
        bias_t = const.tile([MP, 2 * NPIX], MM_DT, name="bias_t")
        # 2-quad-aligned chunks so a quad's bias-add never straddles an
        # unconverted chunk boundary.
        BIAS_CHUNKS = [(fh, c * 2 * QPX, min((c + 1) * 2 * QPX, NPIX))
                       for c in range(4) for fh in range(2)]

        def load_consts_late():
            nc.scalar.dma_start(w2_t[:], w2[:])
            for fh, lo, hi in BIAS_CHUNKS:
                nc.scalar.dma_start(
                    bias8_t[:, fh * NPIX + lo:fh * NPIX + hi],
                    bias[:, fh * NPIX + lo:fh * NPIX + hi])

        conv_cursor = [0]

        def convert_bias(n):
            for fh, lo, hi in BIAS_CHUNKS[conv_cursor[0]:conv_cursor[0] + n]:
                nc.vector.tensor_scalar_add(
                    bias_t[:, fh * NPIX + lo:fh * NPIX + hi],
                    bias8_t[:, fh * NPIX + lo:fh * NPIX + hi], 0.0)
            conv_cursor[0] += n

        # HAM warm-up: bridge the PE from ~t=0 until x0's first rows + w1
        # land so the clock-gate ramp counter never resets.
        dummy = const.tile([MP, 512], MM_DT, name="dummy")
        nc.gpsimd.memset(dummy[:], 0.0)
        for _ in range(N_WARMUP_MM):
            psd = ps1pool.tile([MP, 1024], F32, name="psd", tag="psAB")
            nc.tensor.matmul(psd[:, 0:512], lhsT=dummy[:, 0:MP],
                             rhs=dummy[:],
                             start=True, stop=True, skip_group_check=True)
        load_consts_late()

        def load_image(img, chunked=False):
            """bufA = [x ; x shifted 1 col] via HBM loads on the sync
            queue (kept exclusive to x so loads never queue behind other
            traffic; the early fabric is aggregate-bandwidth-bound, so
            spreading x over more queues does not help). Image 0 loads in
            2 row-chunks so quad 0 starts as early as the fabric allows.
            The lo tail element stays garbage: it is never read."""
            bufa = xpool.tile([2 * C, XL], MM_DT, name="bufa", tag="bufa")
            if chunked:
                cut = X0_ROWS * WP
                nc.sync.dma_start(bufa[0:C, 0:cut], xt[img, :, 0:cut])
                nc.sync.dma_start(bufa[C:2 * C, 0:cut],
                                  xt[img, :, 1:cut + 1])
                nc.sync.dma_start(bufa[0:C, cut:XL], xt[img, :, cut:XL])
                nc.sync.dma_start(bufa[C:2 * C, cut:XL - 1],
                                  xt[img, :, cut + 1:XL])
            else:
                nc.sync.dma_start(bufa[0:C, :], xt[img])
                nc.sync.dma_start(bufa[C:2 * C, 0:XL - 1],
                                  xt[img, :, 1:XL])
            return bufa

        def stage1_quad(bufa, q):
            """Conv 576->100 for quad q (row-pairs 2q, 2q+1; 896 px);
            returns the per-quad z1 tile (drained on ACT)."""
            av = bufa[:, 0:XL].rearrange("c (r w) -> c r w", w=WP)
            rA = 4 * q
            rB = 4 * q + 2
            psAB = ps1pool.tile([MP, 1024], F32, name="psAB", tag="psAB")
            psA = psAB[:, 0:RPP]
            psB = psAB[:, 512:512 + RPP]
            for p in range(3):  # shift pairs (p,0)+(p,1), K=128
                lw = w1_t[:, p * MP:(p + 1) * MP]
                nc.tensor.matmul(psA, lhsT=lw,
                                 rhs=av[:, rA + p:rA + p + 2, 0:W],
                                 start=(p == 0), stop=False,
                                 skip_group_check=True)
                nc.tensor.matmul(psB, lhsT=lw,
                                 rhs=av[:, rB + p:rB + p + 2, 0:W],
                                 start=(p == 0), stop=False,
                                 skip_group_check=True)
            for s in range(3):  # singles (s,2), K=64, concurrent row-tiles
                sl = (3 + s) * MP
                last = (s == 2)
                nc.tensor.matmul(psA, lhsT=w1_t[0:C, sl:sl + MP],
                                 rhs=av[0:C, rA + s:rA + s + 2, 2:2 + W],
                                 start=False, stop=last,
                                 skip_group_check=True, tile_position=(0, 0))
                nc.tensor.matmul(psB, lhsT=w1_t[C:2 * C, sl:sl + MP],
                                 rhs=av[C:2 * C, rB + s:rB + s + 2, 1:1 + W],
                                 start=False, stop=last,
                                 skip_group_check=True, tile_position=(C, 0))
            z1t = z1pool.tile([RANK, QPX], MM_DT, name="z1", tag="z1")
            zsrc = psAB[0:RANK, :].rearrange("r (a b) -> r a b", b=512)
            nc.scalar.activation(
                z1t[:].rearrange("r (a b) -> r a b", b=RPP),
                zsrc[:, :, 0:RPP], fcopy)
            return z1t

        ogs = {}  # (img, fh) -> og tile

        def get_og(img, fh):
            if (img, fh) not in ogs:
                ogs[(img, fh)] = ogpool.tile([MP, NPIX], MM_DT, name="og",
                                             tag="og")
            return ogs[(img, fh)]

        def s2_psum(fh, z1t):
            lw2 = w2_t[:, fh * MP:(fh + 1) * MP]
            ps2 = ps2pool.tile([MP, 1024], F32, name="ps2", tag="ps2")
            for half in range(2):
                nc.tensor.matmul(ps2[:, 512 * half:512 * half + RPP],
                                 lhsT=lw2,
                                 rhs=z1t[:, half * RPP:(half + 1) * RPP],
                                 start=True, stop=True,
                                 skip_group_check=True)
            return ps2

        def s2_quad(img, q, z1t):
            """100->256 + epilogue for one drained quad (steady state)."""
            for fh in range(2):
                ps2 = s2_psum(fh, z1t)
                og = get_og(img, fh)
                base = q * QPX
                oc = og[:, base:base + QPX]
                bc = bias_t[:, fh * NPIX + base:fh * NPIX + base + QPX]
                nc.vector.tensor_tensor(
                    oc.rearrange("f (a b) -> f a b", b=RPP),
                    ps2[:].rearrange("f (a b) -> f a b", b=512)[:, :, 0:RPP],
                    bc.rearrange("f (a b) -> f a b", b=RPP), add)
                nc.vector.tensor_scalar_max(oc, oc, 0.0)
                deng = nc.gpsimd if fh == 0 else nc.scalar
                if q == 2:
                    deng.dma_start(out[img, fh, :, 0:SCUT], og[:, 0:SCUT])
                elif q == NQ - 1:
                    deng.dma_start(out[img, fh, :, SCUT:NPIX],
                                   og[:, SCUT:NPIX])
                    del ogs[(img, fh)]

        DENGS = [nc.gpsimd, nc.scalar, nc.sync]

        def s2_quad_last(img, q, z1t):
            """Last image: per-quad immediate relu + store on rotating
            queues; fh1 takes the ACT-copy path (ACT's drain duty is
            ending) so DVE keeps up with the PE to the very end."""
            for fh in range(2):
                ps2 = s2_psum(fh, z1t)
                og = get_og(img, fh)
                base = q * QPX
                oc = og[:, base:base + QPX]
                bc = bias_t[:, fh * NPIX + base:fh * NPIX + base + QPX]
                psv = ps2[:].rearrange("f (a b) -> f a b", b=512)[:, :, 0:RPP]
                ocv = oc.rearrange("f (a b) -> f a b", b=RPP)
                bcv = bc.rearrange("f (a b) -> f a b", b=RPP)
                if fh == 1:
                    t2 = t2pool.tile([MP, 1024], MM_DT, name="t2", tag="t2")
                    t2v = t2[:].rearrange("f (a b) -> f a b",
                                          b=512)[:, :, 0:RPP]
                    nc.scalar.activation(t2v, psv, fcopy)
                    nc.vector.tensor_tensor(ocv, t2v, bcv, add)
                else:
                    nc.vector.tensor_tensor(ocv, psv, bcv, add)
                nc.vector.tensor_scalar_max(oc, oc, 0.0)
                if q >= NQ - 2:
                    # final quads: two 448-px stores each so the last DMA
                    # flush is small
                    for hh in range(2):
                        DENGS[(2 * q + 2 * fh + hh) % 3].dma_start(
                            out[img, fh, :, base + hh * RPP:
                                base + (hh + 1) * RPP],
                            og[:, base + hh * RPP:base + (hh + 1) * RPP])
                else:
                    DENGS[(2 * q + fh) % 3].dma_start(
                        out[img, fh, :, base:base + QPX], oc)
                if q == NQ - 1:
                    del ogs[(img, fh)]

        # ---- schedule ----
        ready = []  # (img, q, z1t) drained, not yet through stage-2
        bufa_cur = load_image(0, chunked=True)
        for img in range(B):
            bufa_next = load_image(img + 1) if img + 1 < B else None
            last = (img == B - 1)
            z1_prev = None
            for q in range(NQ):
                z1t = stage1_quad(bufa_cur, q)
                if img == 0 and 1 <= q <= 4:
                    convert_bias(2)
                if not last:
                    ready.append((img, q, z1t))
                    take = len(ready) - 1
                    if take > 0:
                        for e in ready[:take]:
                            s2_quad(*e)
                        del ready[:take]
                else:
                    for e in ready:
                        s2_quad(*e)
                    ready.clear()
                    if q > 0:
                        s2_quad_last(img, q - 1, z1_prev)
                    z1_prev = z1t
            bufa_cur = bufa_next
        s2_quad_last(B - 1, NQ - 1, z1_prev)  # trailing quad
        # warm-down: keep the PE HAM clock-gate open while the last
        # epilogue ops + stores flush (engines run ~10% slower at k=4).
        for _ in range(N_WARMDOWN_MM):
            psd = ps1pool.tile([MP, 1024], F32, name="psd", tag="psAB")
            nc.tensor.matmul(psd[:, 0:512], lhsT=dummy[:, 0:MP],
                             rhs=dummy[:],
                             start=True, stop=True, skip_group_check=True)

    nc.compile()
    return nc


def _get_nc():
    if "nc" not in _CACHE:
        _CACHE["nc"] = _build_nc()
    return _CACHE["nc"]


def _prep_inputs(x, k, l_t, s, aux_U, aux_Unp1, aux_Vt, aux_Vtnp1, b, aux_b,
                 step):
    step = int(np.asarray(step))
    x = np.ascontiguousarray(np.asarray(x, dtype=np.float32))
    if step == 0:
        U, W2, bias = np.asarray(k), np.asarray(aux_Vt), np.asarray(aux_b)
    elif step == 1:
        U, W2, bias = np.asarray(aux_U), np.asarray(l_t), np.asarray(aux_b)
    else:
        U = np.asarray(aux_Unp1)
        W2 = (np.asarray(s, np.float64) @ np.asarray(aux_Vtnp1, np.float64))
        bias = np.asarray(b)
    U = U.astype(np.float32)
    W2 = np.ascontiguousarray(W2.astype(MM_NP))
    bias = np.asarray(bias, np.float32)

    # channel-major, zero-padded H and W, fp16
    xpad = np.zeros((B, H + 2, W + 2, C), np.float32)
    xpad[:, 1:-1, 1:-1, :] = x
    xpad_t = np.ascontiguousarray(xpad.transpose(0, 3, 1, 2)).astype(MM_NP)

    # stage-1 stationary slots [128, 6*128]:
    #   p=0..2: top=blocks[p,0], bottom=blocks[p,1] (pairs, K=128)
    #   p=3..5: blocks[p-3,2] duplicated into both halves (concurrent K=64
    #           row-tiles for row-pairs A and B)
    blocks = U.reshape(KH, KW, C, RANK)
    w1p = np.zeros((6, 2 * C, MP), np.float32)
    for p in range(3):
        w1p[p, 0:C, 0:RANK] = blocks[p, 0]
        w1p[p, C:2 * C, 0:RANK] = blocks[p, 1]
    for s_ in range(3):
        w1p[3 + s_, 0:C, 0:RANK] = blocks[s_, 2]
        w1p[3 + s_, C:2 * C, 0:RANK] = blocks[s_, 2]
    w1 = np.ascontiguousarray(
        w1p.transpose(1, 0, 2).reshape(2 * C, 6 * MP)).astype(MM_NP)

    in_maps = []
    for i in range(NCORES):
        xt_i = np.ascontiguousarray(
            xpad_t[:, :, HS * i:HS * i + HSH, :]).reshape(B, C, XL)
        # bias strip -> [f, fh*NPIX + px] (transposed, filter-major, fp8)
        bs = bias[HS * i:HS * (i + 1)].reshape(NPIX, FILTERS)
        bt = np.ascontiguousarray(bs.T)                    # (256, NPIX)
        b_i = np.ascontiguousarray(
            np.concatenate([bt[0:MP], bt[MP:FILTERS]],
                           axis=1)).astype(ml_dtypes.float8_e4m3)
        in_maps.append({"xt": xt_i, "w1": w1, "w2": W2, "bias": b_i})
    return in_maps


def _assemble(results):
    strips = [
        results[i]["out"].transpose(0, 3, 1, 2).reshape(B, HS, W, FILTERS)
        for i in range(NCORES)
    ]
    return np.concatenate(strips, axis=1).astype(np.float32)


def run(trace=False, **inputs):
    in_maps = _prep_inputs(**inputs)
    nc = _get_nc()
    res = run_bass_kernel_spmd(nc, in_maps, list(range(NCORES)), trace=trace)
    return _assemble(res.results), res


def kernel(**inputs):
    out, _ = run(trace=False, **inputs)
    return out


# revision 19
# speedup vs baseline: 1.2162x; 1.2162x over previous
"""Trainium2 Bass kernel for DLRA conv layer (3x3 low-rank conv + bias + relu).

Computes: relu(extract_patches_3x3(x) @ U @ W2 + bias) for the step-selected
factor set (W2 = S @ V folded on host for step 2). Sharded over H across 8
NeuronCores (28 rows each, 1-px halo resolved on host).

Device dataflow per core, per image:
  stage 1 (576->100): per quad (2 row-pairs A/B = 4 output rows, 896 px):
    - 3 full-array K=128 matmuls per row-pair contract shift pairs
      (di,0)+(di,1) via bufA = [x ; x<<1col] (2 HBM loads, no SBUF shift).
    - the 3 leftover dj=2 singles are K=64 and run as CONCURRENT row-tiled
      matmul pairs (tile_position (0,0)/(64,0)) -> 9 PE slots per 896 px.
    - ACT drains psum -> per-quad z1 tiles (fp16, [100 x 896]). ACT does
      NOTHING else in steady state: a busy ACT delays drains, drains gate
      stage-2, and stage-2 stalls are PE stalls (measured as the dominant
      mid-stream loss when ACT carried extra copies).
  stage 2 (100->256): one psum tile [128 x 1024] per (quad, filter-half),
    two N=448 matmuls at cols 0:448 / 512:960 (a matmul may not cross a
    psum bank). A ready-queue of drained quads is consumed down to a
    1-quad lag after each quad's stage-1.
  epilogue: bias-add reads PSUM directly on DVE (the only non-ACT engine
    with PSUM read access; gpsimd's vector ops are ~30x slower), then an
    in-place per-quad fp16 relu at DVE 4x mode. og strips store per fh in
    2 chunks (gpsimd queue for fh0, scalar queue for fh1). DVE runs ~95%
    busy; that is fine because the ps2 ring only needs an add every
    ~2.4us and DVE never falls a full tile behind.
  last image: per-quad stage-2 with immediate relu + store on rotating
    DMA queues; its fh1 tiles take the ACT fp32->fp16 copy + DVE 2x sbuf
    add path (ACT has slack once its drains are ending) so the post-
    matmul tail is one small tile deep (~4us instead of ~10us).

  Scheduling and DMA: each hardware DMA queue has ~2.6us bring-up and
  ~50-150GB/s effective early bandwidth, so the first ~16us are DMA-bound
  no matter what. x0's first 6 rows ride the sync queue alone, the rest
  splits sync (hi) / gpsimd (lo-shifted); ~26 dummy warm-up matmuls keep
  the PE HAM clock-gate counter alive until real work starts (the 2.4GHz
  clock needs ~3.4us of sustained PE activity, and any idle gap drops it
  to half speed for >=3.4us). w1 loads immediately; w2 + bias transfers
  are emitted behind quad-0's drain on the ACT stream so they cannot
  steal early HBM bandwidth from x0. The bias ships as fp8e4m3 (max abs
  quantization error ~8e-3 vs the 3.9e-2 tolerance) to halve its DMA
  cost; DVE upconverts it to fp16 chunk-by-chunk while nearly idle
  during image 0. Host transposes [fh,f,px] -> (H,W,256), casts fp32.
"""

import numpy as np
import ml_dtypes
from contextlib import ExitStack

import concourse.bacc as bacc
import concourse.tile as tile
import concourse.mybir as mybir
from concourse.bass_utils import run_bass_kernel_spmd

B, H, W, C = 8, 224, 224, 64
KH = KW = 3
RANK = 100
FILTERS = 256
IN_DIM = KH * KW * C  # 576

NCORES = 8
HS = H // NCORES          # 28 output rows per core
HSH = HS + 2              # input rows incl halo
WP = W + 2                # padded width
XL = HSH * WP             # flat image-strip length per channel (6780)
NPIX = HS * W             # 6272 pixels per image strip
RPP = 2 * W               # 448 px per row-pair (stage-1 matmul N)
QPX = 2 * RPP             # 896 px per quad
NQ = HS // 4              # 7 quads (2 row-pairs each) per image
MP = 128                  # padded stationary columns (rank 100 -> 128)
X0_ROWS = 10              # first-chunk padded rows of image 0
SCUT = 3 * QPX            # 2688: og store-chunk boundary

N_WARMUP_MM = 3
N_WARMDOWN_MM = 0

F32 = mybir.dt.float32
E4 = mybir.dt.float8e4
MM_DT = mybir.dt.float16
MM_NP = np.float16

_CACHE = {}


def _build_nc():
    nc = bacc.Bacc("TRN2", target_bir_lowering=False, debug=False,
                   num_devices=NCORES)
    xt = nc.dram_tensor("xt", [B, C, XL], MM_DT, kind="ExternalInput").ap()
    w1 = nc.dram_tensor("w1", [2 * C, 6 * MP], MM_DT,
                        kind="ExternalInput").ap()
    w2 = nc.dram_tensor("w2", [RANK, FILTERS], MM_DT,
                        kind="ExternalInput").ap()
    bias = nc.dram_tensor("bias", [MP, 2 * NPIX], MM_DT,
                          kind="ExternalInput").ap()
    out = nc.dram_tensor("out", [B, 2, MP, NPIX], MM_DT,
                         kind="ExternalOutput").ap()
    fcopy = mybir.ActivationFunctionType.Copy
    add = mybir.AluOpType.add

    with tile.TileContext(nc) as tc, ExitStack() as ctx:
        const = ctx.enter_context(tc.tile_pool(name="const", bufs=1))
        xpool = ctx.enter_context(tc.tile_pool(name="xpool", bufs=2))
        z1pool = ctx.enter_context(tc.tile_pool(name="z1pool", bufs=4))
        ps1pool = ctx.enter_context(
            tc.tile_pool(name="ps1", bufs=2, space="PSUM"))
        ps2pool = ctx.enter_context(
            tc.tile_pool(name="ps2", bufs=2, space="PSUM"))
        ogpool = ctx.enter_context(tc.tile_pool(name="ogpool", bufs=4))
        t2pool = ctx.enter_context(tc.tile_pool(name="t2pool", bufs=2))

        w1_t = const.tile([2 * C, 6 * MP], MM_DT, name="w1_t")
        nc.scalar.dma_start(w1_t[:], w1[:])
        w2_t = const.tile([RANK, FILTERS], MM_DT, name="w2_t")
        nc.scalar.dma_start(w2_t[:], w2[:])
        bias_t = const.tile([MP, 2 * NPIX], MM_DT, name="bias_t")
        HPX = NPIX // 2
        for fh in range(2):
            nc.scalar.dma_start(bias_t[:, fh * NPIX:fh * NPIX + HPX],
                                bias[:, fh * NPIX:fh * NPIX + HPX])
        for fh in range(2):
            nc.scalar.dma_start(bias_t[:, fh * NPIX + HPX:(fh + 1) * NPIX],
                                bias[:, fh * NPIX + HPX:(fh + 1) * NPIX])

        # HAM warm-up: bridge the PE from ~t=0 until x0's first rows + w1
        # land so the clock-gate ramp counter never resets.
        dummy = const.tile([MP, 512], MM_DT, name="dummy")
        nc.gpsimd.memset(dummy[:], 0.0)
        for _ in range(N_WARMUP_MM):
            psd = ps1pool.tile([MP, 1024], F32, name="psd", tag="psAB")
            nc.tensor.matmul(psd[:, 0:512], lhsT=dummy[:, 0:MP],
                             rhs=dummy[:],
                             start=True, stop=True, skip_group_check=True)

        def load_image(img, chunked=False):
            """bufA = [x ; x shifted 1 col] via HBM loads on the sync
            queue (kept exclusive to x so loads never queue behind other
            traffic; the early fabric is aggregate-bandwidth-bound, so
            spreading x over more queues does not help). Image 0 loads in
            2 row-chunks so quad 0 starts as early as the fabric allows.
            The lo tail element stays garbage: it is never read."""
            bufa = xpool.tile([2 * C, XL], MM_DT, name="bufa", tag="bufa")
            if chunked:
                cut = X0_ROWS * WP
                nc.sync.dma_start(bufa[0:C, 0:cut], xt[img, :, 0:cut])
                nc.sync.dma_start(bufa[C:2 * C, 0:cut],
                                  xt[img, :, 1:cut + 1])
                nc.sync.dma_start(bufa[0:C, cut:XL], xt[img, :, cut:XL])
                nc.sync.dma_start(bufa[C:2 * C, cut:XL - 1],
                                  xt[img, :, cut + 1:XL])
            else:
                nc.sync.dma_start(bufa[0:C, :], xt[img])
                nc.sync.dma_start(bufa[C:2 * C, 0:XL - 1],
                                  xt[img, :, 1:XL])
            return bufa

        def stage1_quad(bufa, q):
            """Conv 576->100 for quad q (row-pairs 2q, 2q+1; 896 px);
            returns the per-quad z1 tile (drained on ACT)."""
            av = bufa[:, 0:XL].rearrange("c (r w) -> c r w", w=WP)
            rA = 4 * q
            rB = 4 * q + 2
            psAB = ps1pool.tile([MP, 1024], F32, name="psAB", tag="psAB")
            psA = psAB[:, 0:RPP]
            psB = psAB[:, 512:512 + RPP]
            for p in range(3):  # shift pairs (p,0)+(p,1), K=128
                lw = w1_t[:, p * MP:(p + 1) * MP]
                nc.tensor.matmul(psA, lhsT=lw,
                                 rhs=av[:, rA + p:rA + p + 2, 0:W],
                                 start=(p == 0), stop=False,
                                 skip_group_check=True)
                nc.tensor.matmul(psB, lhsT=lw,
                                 rhs=av[:, rB + p:rB + p + 2, 0:W],
                                 start=(p == 0), stop=False,
                                 skip_group_check=True)
            for s in range(3):  # singles (s,2), K=64, concurrent row-tiles
                sl = (3 + s) * MP
                last = (s == 2)
                nc.tensor.matmul(psA, lhsT=w1_t[0:C, sl:sl + MP],
                                 rhs=av[0:C, rA + s:rA + s + 2, 2:2 + W],
                                 start=False, stop=last,
                                 skip_group_check=True, tile_position=(0, 0))
                nc.tensor.matmul(psB, lhsT=w1_t[C:2 * C, sl:sl + MP],
                                 rhs=av[C:2 * C, rB + s:rB + s + 2, 1:1 + W],
                                 start=False, stop=last,
                                 skip_group_check=True, tile_position=(C, 0))
            z1t = z1pool.tile([RANK, QPX], MM_DT, name="z1", tag="z1")
            zsrc = psAB[0:RANK, :].rearrange("r (a b) -> r a b", b=512)
            nc.scalar.activation(
                z1t[:].rearrange("r (a b) -> r a b", b=RPP),
                zsrc[:, :, 0:RPP], fcopy)
            return z1t

        ogs = {}  # (img, fh) -> og tile

        def get_og(img, fh):
            if (img, fh) not in ogs:
                ogs[(img, fh)] = ogpool.tile([MP, NPIX], MM_DT, name="og",
                                             tag="og")
            return ogs[(img, fh)]

        def s2_psum(fh, z1t):
            lw2 = w2_t[:, fh * MP:(fh + 1) * MP]
            ps2 = ps2pool.tile([MP, 1024], F32, name="ps2", tag="ps2")
            for half in range(2):
                nc.tensor.matmul(ps2[:, 512 * half:512 * half + RPP],
                                 lhsT=lw2,
                                 rhs=z1t[:, half * RPP:(half + 1) * RPP],
                                 start=True, stop=True,
                                 skip_group_check=True)
            return ps2

        def s2_quad(img, q, z1t):
            """100->256 + epilogue for one drained quad (steady state)."""
            for fh in range(2):
                ps2 = s2_psum(fh, z1t)
                og = get_og(img, fh)
                base = q * QPX
                oc = og[:, base:base + QPX]
                bc = bias_t[:, fh * NPIX + base:fh * NPIX + base + QPX]
                nc.vector.tensor_tensor(
                    oc.rearrange("f (a b) -> f a b", b=RPP),
                    ps2[:].rearrange("f (a b) -> f a b", b=512)[:, :, 0:RPP],
                    bc.rearrange("f (a b) -> f a b", b=RPP), add)
                nc.vector.tensor_scalar_max(oc, oc, 0.0)
                deng = nc.gpsimd if fh == 0 else nc.scalar
                if img == B - 1:
                    # last image: store per quad so the tail is short
                    deng.dma_start(out[img, fh, :, base:base + QPX], oc)
                    if q == NQ - 1:
                        del ogs[(img, fh)]
                elif q == 2:
                    deng.dma_start(out[img, fh, :, 0:SCUT], og[:, 0:SCUT])
                elif q == NQ - 1:
                    deng.dma_start(out[img, fh, :, SCUT:NPIX],
                                   og[:, SCUT:NPIX])
                    del ogs[(img, fh)]

        DENGS = [nc.gpsimd, nc.scalar, nc.sync]

        def s2_quad_last(img, q, z1t):
            """Last image: per-quad immediate relu + store on rotating
            queues; fh1 takes the ACT-copy path (ACT's drain duty is
            ending) so DVE keeps up with the PE to the very end."""
            for fh in range(2):
                ps2 = s2_psum(fh, z1t)
                og = get_og(img, fh)
                base = q * QPX
                oc = og[:, base:base + QPX]
                bc = bias_t[:, fh * NPIX + base:fh * NPIX + base + QPX]
                psv = ps2[:].rearrange("f (a b) -> f a b", b=512)[:, :, 0:RPP]
                ocv = oc.rearrange("f (a b) -> f a b", b=RPP)
                bcv = bc.rearrange("f (a b) -> f a b", b=RPP)
                if fh == 1:
                    t2 = t2pool.tile([MP, 1024], MM_DT, name="t2", tag="t2")
                    t2v = t2[:].rearrange("f (a b) -> f a b",
                                          b=512)[:, :, 0:RPP]
                    nc.scalar.activation(t2v, psv, fcopy)
                    nc.vector.tensor_tensor(ocv, t2v, bcv, add)
                else:
                    nc.vector.tensor_tensor(ocv, psv, bcv, add)
                nc.vector.tensor_scalar_max(oc, oc, 0.0)
                if q >= NQ - 2:
                    # final quads: two 448-px stores each so the last DMA
                    # flush is small
                    for hh in range(2):
                        DENGS[(2 * q + 2 * fh + hh) % 3].dma_start(
                            out[img, fh, :, base + hh * RPP:
                                base + (hh + 1) * RPP],
                            og[:, base + hh * RPP:base + (hh + 1) * RPP])
                else:
                    DENGS[(2 * q + fh) % 3].dma_start(
                        out[img, fh, :, base:base + QPX], oc)
                if q == NQ - 1:
                    del ogs[(img, fh)]

        # ---- schedule ----
        ready = []  # (img, q, z1t) drained, not yet through stage-2
        bufa_cur = load_image(0, chunked=True)
        for img in range(B):
            bufa_next = load_image(img + 1) if img + 1 < B else None
            for q in range(NQ):
                z1t = stage1_quad(bufa_cur, q)
                ready.append((img, q, z1t))
                take = len(ready) - 1
                if take > 0:
                    for e in ready[:take]:
                        s2_quad(*e)
                    del ready[:take]
            bufa_cur = bufa_next
        for e in ready:  # trailing quad of the last image
            s2_quad(*e)
        ready.clear()
        # warm-down: keep the PE HAM clock-gate open while the last
        # epilogue ops + stores flush (engines run ~10% slower at k=4).
        for _ in range(N_WARMDOWN_MM):
            psd = ps1pool.tile([MP, 1024], F32, name="psd", tag="psAB")
            nc.tensor.matmul(psd[:, 0:512], lhsT=dummy[:, 0:MP],
                             rhs=dummy[:],
                             start=True, stop=True, skip_group_check=True)

    nc.compile()
    return nc


def _get_nc():
    if "nc" not in _CACHE:
        _CACHE["nc"] = _build_nc()
    return _CACHE["nc"]


def _prep_inputs(x, k, l_t, s, aux_U, aux_Unp1, aux_Vt, aux_Vtnp1, b, aux_b,
                 step):
    step = int(np.asarray(step))
    x = np.ascontiguousarray(np.asarray(x, dtype=np.float32))
    if step == 0:
        U, W2, bias = np.asarray(k), np.asarray(aux_Vt), np.asarray(aux_b)
    elif step == 1:
        U, W2, bias = np.asarray(aux_U), np.asarray(l_t), np.asarray(aux_b)
    else:
        U = np.asarray(aux_Unp1)
        W2 = (np.asarray(s, np.float64) @ np.asarray(aux_Vtnp1, np.float64))
        bias = np.asarray(b)
    U = U.astype(np.float32)
    W2 = np.ascontiguousarray(W2.astype(MM_NP))
    bias = np.asarray(bias, np.float32)

    # channel-major, zero-padded H and W, fp16
    xpad = np.zeros((B, H + 2, W + 2, C), np.float32)
    xpad[:, 1:-1, 1:-1, :] = x
    xpad_t = np.ascontiguousarray(xpad.transpose(0, 3, 1, 2)).astype(MM_NP)

    # stage-1 stationary slots [128, 6*128]:
    #   p=0..2: top=blocks[p,0], bottom=blocks[p,1] (pairs, K=128)
    #   p=3..5: blocks[p-3,2] duplicated into both halves (concurrent K=64
    #           row-tiles for row-pairs A and B)
    blocks = U.reshape(KH, KW, C, RANK)
    w1p = np.zeros((6, 2 * C, MP), np.float32)
    for p in range(3):
        w1p[p, 0:C, 0:RANK] = blocks[p, 0]
        w1p[p, C:2 * C, 0:RANK] = blocks[p, 1]
    for s_ in range(3):
        w1p[3 + s_, 0:C, 0:RANK] = blocks[s_, 2]
        w1p[3 + s_, C:2 * C, 0:RANK] = blocks[s_, 2]
    w1 = np.ascontiguousarray(
        w1p.transpose(1, 0, 2).reshape(2 * C, 6 * MP)).astype(MM_NP)

    in_maps = []
    for i in range(NCORES):
        xt_i = np.ascontiguousarray(
            xpad_t[:, :, HS * i:HS * i + HSH, :]).reshape(B, C, XL)
        # bias strip -> [f, fh*NPIX + px] (transposed, filter-major, fp8)
        bs = bias[HS * i:HS * (i + 1)].reshape(NPIX, FILTERS)
        bt = np.ascontiguousarray(bs.T).astype(MM_NP)      # (256, NPIX)
        b_i = np.ascontiguousarray(
            np.concatenate([bt[0:MP], bt[MP:FILTERS]], axis=1))
        in_maps.append({"xt": xt_i, "w1": w1, "w2": W2, "bias": b_i})
    return in_maps


def _assemble(results):
    strips = [
        results[i]["out"].transpose(0, 3, 1, 2).reshape(B, HS, W, FILTERS)
        for i in range(NCORES)
    ]
    return np.concatenate(strips, axis=1).astype(np.float32)


def run(trace=False, **inputs):
    in_maps = _prep_inputs(**inputs)
    nc = _get_nc()
    res = run_bass_kernel_spmd(nc, in_maps, list(range(NCORES)), trace=trace)
    return _assemble(res.results), res


def kernel(**inputs):
    out, _ = run(trace=False, **inputs)
    return out
